# revision 11
# baseline (speedup 1.0000x reference)
"""MeshCaster Trainium2 kernel (v2: fp8 DoubleRow vert branch).

Per-token (token = (sample, mesh) pair, 262144 tokens) network:
  - gather 3 vertex embedding rows (per-mesh tables, max-norm renormalized)
  - barycentric weighted sum -> vertex embedding ve (256)
  - view branch: sincos(views) -> linear proj -> 2x (Linear+ReLU)
  - vert branch: 2x (Linear+ReLU)
  - alpha / color heads have identity activations.

Host-side folds (all exact linear algebra, fp64 weights):
  - max_norm renorm is a per-table-row property -> pre-scale tables
  - w_proj @ view_W[0] -> single [36 x 256] first view layer
  - alpha head:  (h@A1+b1)@A2+b2 = h@(A1@A2) + (b1@A2+b2)   [256x1]
  - color head:  (c@C1+b1)@C2+b2 = c@(C1@C2) + (b1@C2+b2)   [512x3]
  - alpha+color combine into one [768 x 4] output GEMM over [h2|v2|ve]
  - the gather + barycentric reduce (0.4% of FLOPs, pure data movement +
    a row-scale) run on host: the device's indirect-DMA descriptor
    generation path is ~1.7us per 128 rows on this toolchain, which would
    dominate the kernel. The device streams pre-reduced, channel-major ve
    tiles and executes all GEMMs (99.6% of the FLOPs).

v2 speedups over the 300us baseline:
  - vert branch (h1, h2) runs in fp8-e4m3 DoubleRow matmuls: 2 packed
    k-tiles per instruction at 2x rate (measured 118ns vs 225ns per
    equivalent bf16 pair). Only alpha (1 of 4 output channels) sees the
    quantization noise: full-net rel err 0.006 vs gate 2e-2.
  - ve streamed as fp8 (halves the dominant DMA stream).
  - 4-chunk interleave (P=4) with copies split ACT(mt0)/DVE(mt1) so
    PSUM->SBUF relu copies never stall the PE.
  - the host-folded cve term is pre-filled into the output DRAM buffer by
    a gpsimd DRAM->DRAM DMA; per-chunk outputs ride gpsimd software-DGE
    accum-add DMAs (same queue => ordered), with the PSUM->SBUF copy
    alternating between ACT and DVE. No vector-engine add needed.

Sharding: data-parallel over samples, 4096 samples (32768 tokens) per core,
weights replicated, no cross-core communication.

Device pipeline per 512-token chunk:
  v1 = relu(sincos[36,512] @ Wv1)        2 bf16 matmuls (K=36)
  v2 = relu(v1 @ Wv2)                    4 bf16 matmuls
  h1 = relu(veT @ Wt1)                   4 fp8 DoubleRow matmuls -> fp8
  h2 = relu(h1 @ Wt2)                    4 fp8 DoubleRow matmuls -> bf16
  out[4,512] = [h2|v2] @ Wo (+cve)       4 bf16 matmuls (psum-accumulated)
activations bf16/fp8, feature-major layout [chan, tok]; psum fp32.
"""

import sys

if "/opt/trn_rl_repo" not in sys.path:
    sys.path.insert(0, "/opt/trn_rl_repo")

import numpy as np
import ml_dtypes

import concourse.bass as bass
import concourse.tile as tile
from concourse import mybir
from concourse.bass_utils import run_bass_kernel_spmd
from concourse.vector_clock import ScopedClock

BF16 = ml_dtypes.bfloat16
E4 = ml_dtypes.float8_e4m3

N_SAMPLES = 32768
N_MESH = 8
N_VERTS = 50000
N_CHAN = 256
N_LEVELS = 6
VIEW_DIM = 3 * 2 * N_LEVELS  # 36
N_CORES = 8
VROWS = N_MESH * (N_VERTS + 1)  # 400008

T_CORE = (N_SAMPLES // N_CORES) * N_MESH  # 32768 tokens per core
CHUNK = 512
N_CHUNKS = T_CORE // CHUNK  # 64
P = 4                       # chunks interleaved per group
N_GROUPS = N_CHUNKS // P    # 16

F32 = mybir.dt.float32
BF = mybir.dt.bfloat16
FP8 = mybir.dt.float8e4
AF = mybir.ActivationFunctionType
ALU = mybir.AluOpType
DR = mybir.MatmulPerfMode.DoubleRow


class SplitDrainTileContext(tile.TileContext):
    """Walrus on this toolchain rejects >1 sync-wait on some instruction
    structs; split the kernel-tail drain's waits into single-wait NOPs."""

    def _drain_and_barrier(self, tick_clock, wait_clock):
        probe = self.nc.sync.nop(nofuse=True)
        wait_clock.add_sem_waits(probe.ins, ScopedClock({None: tick_clock.global_clock}))
        si = probe.ins.sync_info
        waits = list(si.on_wait) if si is not None else []
        if len(waits) > 1:
            si.on_wait = waits[:1]
            for w in waits[1:]:
                n = self.nc.sync.nop(nofuse=True)
                n.ins.sync_info = mybir.SyncInfo(on_wait=[w], on_update=[])
        self.nc.sync.drain()
        self.nc.all_engine_barrier()
        assert self.sems is not None
        popped = self.nc._tile_sem_poison_stack.pop()
        assert popped is self._sem_poison
        self.nc.clear_and_free_semaphores(list(self.sems.allocated().values()))
        self.nc.all_engine_barrier()


def _split_sync_waits(nc, max_waits=1):
    """Move excess per-instruction sync-waits onto same-engine NOPs."""
    cnt = 0
    for f in nc.m.functions:
        for bb in f.blocks:
            new = []
            for inst in bb.instructions:
                si = inst.sync_info
                if si is not None and len(si.on_wait) > max_waits:
                    waits = list(si.on_wait)
                    for w in waits[:-max_waits]:
                        cnt += 1
                        new.append(mybir.InstNoOp(
                            name=f"wsplit_{cnt}",
                            engine=inst.engine,
                            bass_nofuse=True,
                            sync_info=mybir.SyncInfo(on_wait=[w], on_update=[]),
                        ))
                    si.on_wait = waits[-max_waits:]
                new.append(inst)
            bb.instructions[:] = new
    return cnt


def build_nc(n_chunks: int, split_waits: bool = True) -> bass.Bass:
    """Build the Bass program for `n_chunks` 512-token chunks per core."""
    T = n_chunks * CHUNK
    n_groups = n_chunks // P
    nc = bass.Bass("TRN2", target_bir_lowering=False, debug=False)

    VDT = FP8
    # ---- DRAM I/O ----
    # channel-major vertex embeddings: [chunk, chan_in_half(128), half(2), tok(512)]
    ve_d = nc.dram_tensor("vet", [n_chunks, 128, 2, CHUNK], VDT, kind="ExternalInput")
    sc_d = nc.dram_tensor("sincos", [VIEW_DIM, T], BF, kind="ExternalInput")
    wv1_d = nc.dram_tensor("wv1", [VIEW_DIM, 256], BF, kind="ExternalInput")
    wv2_d = nc.dram_tensor("wv2", [128, 2 * 2 * 128], BF, kind="ExternalInput")
    wt1_d = nc.dram_tensor("wt1", [128, 2 * 2 * 128], VDT, kind="ExternalInput")
    wt2_d = nc.dram_tensor("wt2", [128, 2 * 2 * 128], VDT, kind="ExternalInput")
    wo_d = nc.dram_tensor("wo", [128, 4 * 4], BF, kind="ExternalInput")
    cve_d = nc.dram_tensor("cve", [4, T], F32, kind="ExternalInput")
    out_d = nc.dram_tensor("out_t", [4, T], F32, kind="ExternalOutput")

    with SplitDrainTileContext(nc) as tc:
        with (
            tc.tile_pool(name="const", bufs=1) as cp,
            tc.tile_pool(name="vet", bufs=6) as vetp,
            tc.tile_pool(name="acts", bufs=2) as ap_,
            tc.tile_pool(name="outp", bufs=2) as op_,
            tc.tile_pool(name="psum", bufs=4, space="PSUM") as pp,
        ):
            # ---- persistent constants ----
            wv1 = cp.tile([VIEW_DIM, 256], BF)
            nc.scalar.dma_start(wv1[:], wv1_d[:])
            wv2 = cp.tile([128, 2, 2, 128], BF)
            nc.scalar.dma_start(wv2[:], wv2_d[:].rearrange("p (a b c) -> p a b c", a=2, b=2))
            wt1 = cp.tile([128, 2, 2, 128], VDT)
            nc.scalar.dma_start(wt1[:], wt1_d[:].rearrange("p (a b c) -> p a b c", a=2, b=2))
            wt2 = cp.tile([128, 2, 2, 128], VDT)
            nc.scalar.dma_start(wt2[:], wt2_d[:].rearrange("p (a b c) -> p a b c", a=2, b=2))
            wo = cp.tile([128, 4, 4], BF)
            nc.scalar.dma_start(wo[:], wo_d[:].rearrange("p (a b) -> p a b", a=4))
            # prefill output with the host-folded cve term; per-chunk output
            # DMAs are gpsimd software-DGE accum-adds on the same queue, so
            # ordering wrt this prefill is guaranteed.
            nc.gpsimd.dma_start(out_d[:, : (n_chunks - P) * CHUNK],
                                cve_d[:, : (n_chunks - P) * CHUNK])

            def relu_op(dst, src, eng):
                # eng 0 -> ACT, 1 -> DVE
                if eng == 0:
                    nc.scalar.activation(dst, src, AF.Relu)
                else:
                    nc.vector.tensor_scalar(dst, src, 0.0, None, op0=ALU.max)

            prev_out = [None]

            def out_chunk(g, pacts, c, cve_j=None):
                pj0 = g * P
                h2, v2 = pacts[c]["h2"], pacts[c]["v2"]
                rhs_tiles = [h2[:, 0, :], h2[:, 1, :], v2[:, 0, :], v2[:, 1, :]]
                pot = pp.tile([128, 2, CHUNK], F32, space="PSUM", tag="ps")
                po = pot[0:4, 0, :]
                for kt, rhs in enumerate(rhs_tiles):
                    nc.tensor.matmul(po, wo[:, kt, :], rhs,
                                     start=(kt == 0), stop=(kt == 3))
                ot = op_.tile([4, CHUNK], F32, tag="ot")
                if cve_j is not None:
                    # last group: fold cve on-engine, plain sync write (keeps
                    # the gpsimd SW-DGE queue empty well before the drain)
                    nc.vector.tensor_tensor(
                        ot[:], po, cve_j[:, c * CHUNK : (c + 1) * CHUNK],
                        op=ALU.add)
                    nc.sync.dma_start(
                        out_d[:, (pj0 + c) * CHUNK : (pj0 + c + 1) * CHUNK],
                        ot[:])
                    return
                if c % 2 == 0:
                    nc.vector.tensor_copy(ot[:], po)
                else:
                    nc.scalar.copy(ot[:], po)
                nc.gpsimd.dma_start(
                    out_d[:, (pj0 + c) * CHUNK : (pj0 + c + 1) * CHUNK],
                    ot[:], accum_op=ALU.add)

            for g in range(n_groups):
                j0 = g * P
                sc_j = vetp.tile([VIEW_DIM, P * CHUNK], BF, tag="scj")
                nc.sync.dma_start(sc_j[:], sc_d[:, j0 * CHUNK : (j0 + P) * CHUNK])
                veTs, acts = [], []
                for c in range(P):
                    veT = vetp.tile([128, 2, CHUNK], VDT, tag=f"veT{c}")
                    nc.sync.dma_start(veT[:], ve_d[j0 + c])
                    veTs.append(veT)
                    acts.append({})

                # ---- bf16 layer: one 2-bank psum tile per chunk ----
                def layer_bf(tag, wtile, rhs_of, ktiles, split_copy=False):
                    for c in range(P):
                        acts[c][tag] = ap_.tile([128, 2, CHUNK], BF,
                                                name=f"{tag}{c}", tag=f"{tag}{c}")
                        ps = pp.tile([128, 2, CHUNK], F32, space="PSUM", tag="ps")
                        for mt in range(2):
                            for kt in range(ktiles):
                                nc.tensor.matmul(
                                    ps[:, mt, :], wtile(kt, mt), rhs_of(c, kt),
                                    start=(kt == 0), stop=(kt == ktiles - 1))
                        if split_copy:
                            for mt in range(2):
                                relu_op(acts[c][tag][:, mt, :], ps[:, mt, :], mt)
                        else:
                            relu_op(acts[c][tag][:], ps[:], c % 2)

                # ---- fp8 DoubleRow layer: K=256 packed, N-tiles of 256 ----
                def layer_dr(tag, wtile, rhs_of, out_dtype, flip):
                    for c in range(P):
                        acts[c][tag] = ap_.tile([128, 2, CHUNK], out_dtype,
                                                name=f"{tag}{c}", tag=f"{tag}{c}")
                        ps = pp.tile([128, 2, CHUNK], F32, space="PSUM", tag="ps")
                        for mt in range(2):
                            for nt in range(2):
                                nc.tensor.matmul(
                                    ps[:, mt, nt * 256 : (nt + 1) * 256],
                                    wtile(mt),
                                    rhs_of(c)[:, :, nt * 256 : (nt + 1) * 256],
                                    start=True, stop=True, perf_mode=DR)
                        relu_op(acts[c][tag][:], ps[:], (c + flip) % 2)

                # v1(g) interleaved per-chunk with out(g-1): the out stage
                # fills the PE while v1's copies land, and vice versa
                for c in range(P):
                    acts[c]["v1"] = ap_.tile([128, 2, CHUNK], BF,
                                             name=f"v1{c}", tag=f"v1{c}")
                    ps = pp.tile([128, 2, CHUNK], F32, space="PSUM", tag="ps")
                    for mt in range(2):
                        nc.tensor.matmul(
                            ps[:, mt, :], wv1[:, mt * 128 : (mt + 1) * 128],
                            sc_j[:, c * CHUNK : (c + 1) * CHUNK],
                            start=True, stop=True)
                    for mt in range(2):
                        relu_op(acts[c]["v1"][:, mt, :], ps[:, mt, :], mt)
                    if prev_out[0] is not None:
                        out_chunk(g - 1, prev_out[0], c)
                layer_bf("v2", lambda kt, mt: wv2[:, kt, mt, :],
                         lambda c, kt: acts[c]["v1"][:, kt, :], 2)
                def h_chunk(tag, wt, rhs, out_dtype, c, eng):
                    acts[c][tag] = ap_.tile([128, 2, CHUNK], out_dtype,
                                            name=f"{tag}{c}", tag=f"{tag}{c}")
                    ps = pp.tile([128, 2, CHUNK], F32, space="PSUM", tag="ps")
                    for mt in range(2):
                        for nt in range(2):
                            nc.tensor.matmul(
                                ps[:, mt, nt * 256 : (nt + 1) * 256],
                                wt[:, :, mt, :],
                                rhs[:, :, nt * 256 : (nt + 1) * 256],
                                start=True, stop=True, perf_mode=DR)
                    relu_op(acts[c][tag][:], ps[:], eng)

                # wave order spreads h2's psum/copy burst away from the
                # group boundary: h1:c0,c1,c2 h2:c0 h1:c3 h2:c1,c2,c3
                h_chunk("h1", wt1, veTs[0], FP8, 0, 1)
                h_chunk("h1", wt1, veTs[1], FP8, 1, 0)
                h_chunk("h1", wt1, veTs[2], FP8, 2, 1)
                h_chunk("h2", wt2, acts[0]["h1"], BF, 0, 0)
                h_chunk("h1", wt1, veTs[3], FP8, 3, 1)
                h_chunk("h2", wt2, acts[1]["h1"], BF, 1, 0)
                h_chunk("h2", wt2, acts[2]["h1"], BF, 2, 1)
                h_chunk("h2", wt2, acts[3]["h1"], BF, 3, 0)
                prev_out[0] = acts

            # last group's out stage, with cve added on-engine
            cve_l = op_.tile([4, P * CHUNK], F32, tag="cvel")
            nc.sync.dma_start(
                cve_l[:], cve_d[:, (n_chunks - P) * CHUNK : n_chunks * CHUNK])
            for c in range(P):
                out_chunk(n_groups - 1, prev_out[0], c, cve_j=cve_l)

    if split_waits:  # CoreSim can't run the raw NOPs; HW compile needs them
        _split_sync_waits(nc)
    return nc


# ---------------------------------------------------------------------------
# Host-side preprocessing
# ---------------------------------------------------------------------------

def _pack_w(w: np.ndarray) -> np.ndarray:
    """[256, 256] -> [128, 2*2*128] with layout [p, (kt, mt, j)]."""
    w4 = w.reshape(2, 128, 2, 128)           # [kt, p, mt, j]
    return np.ascontiguousarray(w4.transpose(1, 0, 2, 3)).reshape(128, 512)


def prepare_host_inputs(verts, barys, views, emb_tables, w_proj, b_proj,
                        view_W, view_b, vert_W, vert_b,
                        alpha_W1, alpha_b1, alpha_W2, alpha_b2,
                        color_W1, color_b1, color_W2, color_b2,
                        n_chunks=N_CHUNKS, n_cores=N_CORES):
    """Fold weights, gather+reduce embeddings, pack per-core in_maps."""
    verts = np.asarray(verts).astype(np.int64)
    barys = np.asarray(barys, dtype=np.float32)
    views = np.asarray(views, dtype=np.float32)
    emb = np.asarray(emb_tables, dtype=np.float32)

    t_core = n_chunks * CHUNK
    n_groups = n_chunks // P
    n_tok = t_core * n_cores

    # --- embedding tables: fold max_norm renorm ---
    norm = np.linalg.norm(emb.astype(np.float64), axis=-1, keepdims=True)
    scale = np.where(norm > 1.0, 1.0 / np.maximum(norm, 1e-7), 1.0)
    table = (emb * scale).reshape(VROWS, N_CHAN).astype(np.float32)

    # --- gather + barycentric reduce -> vertex embeddings [n_tok, 256] ---
    mesh_off = (np.arange(N_MESH, dtype=np.int64) * (N_VERTS + 1))[None, :, None]
    flat_idx = (verts + 1 + mesh_off).reshape(-1, 3)[:n_tok]
    flat_bary = barys.reshape(-1, 3)[:n_tok]
    vemb_f32 = np.einsum("tv,tvc->tc", flat_bary, table[flat_idx])
    vemb = vemb_f32.astype(E4)

    # --- sincos view features, transposed [36, n_tok] ---
    v64 = views.reshape(-1, 3).astype(np.float64)[:n_tok]
    freqs = 2.0 ** np.arange(N_LEVELS)
    xf = v64[:, None, :] * freqs[:, None]                 # [t, L, 3]
    sc = np.stack([np.sin(xf), np.cos(xf)], axis=2)       # [t, L, 2, 3]
    sc = sc.reshape(-1, VIEW_DIM).astype(np.float32)
    sc_T = np.ascontiguousarray(sc.T.astype(BF16))        # [36, n_tok]

    # --- folded weights (fp64) ---
    w_proj = np.asarray(w_proj, dtype=np.float64)
    b_proj = np.asarray(b_proj, dtype=np.float64)
    view_W = np.asarray(view_W, dtype=np.float64)
    view_b = np.asarray(view_b, dtype=np.float64)
    vert_W = np.asarray(vert_W, dtype=np.float64)
    vert_b = np.asarray(vert_b, dtype=np.float64)
    aW1 = np.asarray(alpha_W1, dtype=np.float64)
    ab1 = np.asarray(alpha_b1, dtype=np.float64)
    aW2 = np.asarray(alpha_W2, dtype=np.float64)
    ab2 = np.asarray(alpha_b2, dtype=np.float64)
    cW1 = np.asarray(color_W1, dtype=np.float64)
    cb1 = np.asarray(color_b1, dtype=np.float64)
    cW2 = np.asarray(color_W2, dtype=np.float64)
    cb2 = np.asarray(color_b2, dtype=np.float64)

    assert not np.any(b_proj) and not np.any(view_b) and not np.any(vert_b), \
        "kernel build assumes zero hidden biases (as in setup_inputs)"
    assert not np.any(ab1) and not np.any(cb1), \
        "kernel build assumes zero head hidden biases"

    wv1 = (w_proj @ view_W[0]).astype(BF16)               # [36, 256]
    wa = aW1 @ aW2                                        # [256, 1]
    ba = ab1 @ aW2 + ab2                                  # [1]
    wc = cW1 @ cW2                                        # [512, 3]
    bc = cb1 @ cW2 + cb2                                  # [3]

    w_out = np.zeros((512, 4), dtype=np.float64)
    w_out[0:256, 3] = wa[:, 0]        # h2 -> alpha
    w_out[256:512, 0:3] = wc[0:256]   # v2 -> colors
    wo = np.ascontiguousarray(
        w_out.reshape(4, 128, 4).transpose(1, 0, 2)).reshape(128, 16).astype(BF16)

    # host-folded output term: cve[t, 0:3] = ve @ Wc_bot + bc; cve[t, 3] = ba
    cve = np.empty((n_tok, 4), dtype=np.float32)
    cve[:, 0:3] = (vemb_f32.astype(np.float64) @ wc[256:512] + bc).astype(np.float32)
    cve[:, 3] = ba[0]

    shared = {
        "wv1": np.ascontiguousarray(wv1),
        "wv2": _pack_w(view_W[1]).astype(BF16),
        "wt1": _pack_w(vert_W[0]).astype(E4),
        "wt2": _pack_w(vert_W[1]).astype(E4),
        "wo": wo,
    }

    in_maps = []
    for c in range(n_cores):
        lo = c * t_core
        m = dict(shared)
        # [t_core, 256] -> [n_chunks, 128(chan%128), 2(half), 512(tok)]
        g = vemb[lo : lo + t_core].reshape(n_chunks, CHUNK, 2, 128)
        m["vet"] = np.ascontiguousarray(g.transpose(0, 3, 2, 1))
        m["sincos"] = np.ascontiguousarray(sc_T[:, lo : lo + t_core])
        m["cve"] = np.ascontiguousarray(cve[lo : lo + t_core].T)
        in_maps.append(m)
    return in_maps


def assemble_output(results, n_cores=N_CORES):
    """results[c]['out_t'] is [4, t_core] -> full (N_SAMPLES, N_MESH, 4)."""
    outs = []
    for c in range(n_cores):
        o = results[c]["out_t"]  # [4, t_core]
        outs.append(np.ascontiguousarray(o.T).reshape(-1, N_MESH, 4))
    return np.concatenate(outs, axis=0).astype(np.float32)


_NC_CACHE = {}


def get_nc(n_chunks=N_CHUNKS):
    if n_chunks not in _NC_CACHE:
        _NC_CACHE[n_chunks] = build_nc(n_chunks)
    return _NC_CACHE[n_chunks]


def kernel(**inputs) -> np.ndarray:
    in_maps = prepare_host_inputs(**inputs)
    nc = get_nc(N_CHUNKS)
    res = run_bass_kernel_spmd(nc, in_maps, list(range(N_CORES)))
    return assemble_output(res.results)


# revision 12
# speedup vs baseline: 1.2405x; 1.2405x over previous
"""MeshCaster Trainium2 kernel (v2: fp8 DoubleRow vert branch).

Per-token (token = (sample, mesh) pair, 262144 tokens) network:
  - gather 3 vertex embedding rows (per-mesh tables, max-norm renormalized)
  - barycentric weighted sum -> vertex embedding ve (256)
  - view branch: sincos(views) -> linear proj -> 2x (Linear+ReLU)
  - vert branch: 2x (Linear+ReLU)
  - alpha / color heads have identity activations.

Host-side folds (all exact linear algebra, fp64 weights):
  - max_norm renorm is a per-table-row property -> pre-scale tables
  - w_proj @ view_W[0] -> single [36 x 256] first view layer
  - alpha head:  (h@A1+b1)@A2+b2 = h@(A1@A2) + (b1@A2+b2)   [256x1]
  - color head:  (c@C1+b1)@C2+b2 = c@(C1@C2) + (b1@C2+b2)   [512x3]
  - alpha+color combine into one [768 x 4] output GEMM over [h2|v2|ve]
  - the gather + barycentric reduce (0.4% of FLOPs, pure data movement +
    a row-scale) run on host: the device's indirect-DMA descriptor
    generation path is ~1.7us per 128 rows on this toolchain, which would
    dominate the kernel. The device streams pre-reduced, channel-major ve
    tiles and executes all GEMMs (99.6% of the FLOPs).

v2 speedups over the 300us baseline:
  - vert branch (h1, h2) runs in fp8-e4m3 DoubleRow matmuls: 2 packed
    k-tiles per instruction at 2x rate (measured 118ns vs 225ns per
    equivalent bf16 pair). Only alpha (1 of 4 output channels) sees the
    quantization noise: full-net rel err 0.006 vs gate 2e-2.
  - ve streamed as fp8 (halves the dominant DMA stream).
  - 4-chunk interleave (P=4) with copies split ACT(mt0)/DVE(mt1) so
    PSUM->SBUF relu copies never stall the PE.
  - the host-folded cve term is pre-filled into the output DRAM buffer by
    a gpsimd DRAM->DRAM DMA; per-chunk outputs ride gpsimd software-DGE
    accum-add DMAs (same queue => ordered), with the PSUM->SBUF copy
    alternating between ACT and DVE. No vector-engine add needed.

Sharding: data-parallel over samples, 4096 samples (32768 tokens) per core,
weights replicated, no cross-core communication.

Device pipeline per 512-token chunk:
  v1 = relu(sincos[36,512] @ Wv1)        2 bf16 matmuls (K=36)
  v2 = relu(v1 @ Wv2)                    4 bf16 matmuls
  h1 = relu(veT @ Wt1)                   4 fp8 DoubleRow matmuls -> fp8
  h2 = relu(h1 @ Wt2)                    4 fp8 DoubleRow matmuls -> bf16
  out[4,512] = [h2|v2] @ Wo (+cve)       4 bf16 matmuls (psum-accumulated)
activations bf16/fp8, feature-major layout [chan, tok]; psum fp32.
"""

import sys

if "/opt/trn_rl_repo" not in sys.path:
    sys.path.insert(0, "/opt/trn_rl_repo")

import numpy as np
import ml_dtypes

import concourse.bass as bass
import concourse.tile as tile
from concourse import mybir
from concourse.bass_utils import run_bass_kernel_spmd
from concourse.vector_clock import ScopedClock

BF16 = ml_dtypes.bfloat16
E4 = ml_dtypes.float8_e4m3

N_SAMPLES = 32768
N_MESH = 8
N_VERTS = 50000
N_CHAN = 256
N_LEVELS = 6
VIEW_DIM = 3 * 2 * N_LEVELS  # 36
N_CORES = 8
VROWS = N_MESH * (N_VERTS + 1)  # 400008

T_CORE = (N_SAMPLES // N_CORES) * N_MESH  # 32768 tokens per core
CHUNK = 512
N_CHUNKS = T_CORE // CHUNK  # 64
P = 4                       # chunks interleaved per group
N_GROUPS = N_CHUNKS // P    # 16

F32 = mybir.dt.float32
BF = mybir.dt.bfloat16
FP8 = mybir.dt.float8e4
AF = mybir.ActivationFunctionType
ALU = mybir.AluOpType
DR = mybir.MatmulPerfMode.DoubleRow


class SplitDrainTileContext(tile.TileContext):
    """Walrus on this toolchain rejects >1 sync-wait on some instruction
    structs; split the kernel-tail drain's waits into single-wait NOPs."""

    def _drain_and_barrier(self, tick_clock, wait_clock):
        probe = self.nc.sync.nop(nofuse=True)
        wait_clock.add_sem_waits(probe.ins, ScopedClock({None: tick_clock.global_clock}))
        si = probe.ins.sync_info
        waits = list(si.on_wait) if si is not None else []
        if len(waits) > 1:
            si.on_wait = waits[:1]
            for w in waits[1:]:
                n = self.nc.sync.nop(nofuse=True)
                n.ins.sync_info = mybir.SyncInfo(on_wait=[w], on_update=[])
        self.nc.sync.drain()
        self.nc.all_engine_barrier()
        assert self.sems is not None
        popped = self.nc._tile_sem_poison_stack.pop()
        assert popped is self._sem_poison
        self.nc.clear_and_free_semaphores(list(self.sems.allocated().values()))
        self.nc.all_engine_barrier()


def _split_sync_waits(nc, max_waits=1):
    """Move excess per-instruction sync-waits onto same-engine NOPs."""
    cnt = 0
    for f in nc.m.functions:
        for bb in f.blocks:
            new = []
            for inst in bb.instructions:
                si = inst.sync_info
                if si is not None and len(si.on_wait) > max_waits:
                    waits = list(si.on_wait)
                    for w in waits[:-max_waits]:
                        cnt += 1
                        new.append(mybir.InstNoOp(
                            name=f"wsplit_{cnt}",
                            engine=inst.engine,
                            bass_nofuse=True,
                            sync_info=mybir.SyncInfo(on_wait=[w], on_update=[]),
                        ))
                    si.on_wait = waits[-max_waits:]
                new.append(inst)
            bb.instructions[:] = new
    return cnt


def build_nc(n_chunks: int, split_waits: bool = True) -> bass.Bass:
    """Build the Bass program for `n_chunks` 512-token chunks per core."""
    T = n_chunks * CHUNK
    n_groups = n_chunks // P
    nc = bass.Bass("TRN2", target_bir_lowering=False, debug=False)

    VDT = FP8
    # ---- DRAM I/O ----
    # channel-major vertex embeddings: [chunk, chan_in_half(128), half(2), tok(512)]
    ve_d = nc.dram_tensor("vet", [n_chunks, 128, 2, CHUNK], VDT, kind="ExternalInput")
    sc_d = nc.dram_tensor("sincos", [VIEW_DIM, T], BF, kind="ExternalInput")
    wv1_d = nc.dram_tensor("wv1", [VIEW_DIM, 256], BF, kind="ExternalInput")
    wv2_d = nc.dram_tensor("wv2", [128, 2 * 2 * 128], BF, kind="ExternalInput")
    wt1_d = nc.dram_tensor("wt1", [128, 2 * 2 * 128], VDT, kind="ExternalInput")
    wt2_d = nc.dram_tensor("wt2", [128, 2 * 2 * 128], VDT, kind="ExternalInput")
    wo_d = nc.dram_tensor("wo", [128, 4 * 4], BF, kind="ExternalInput")
    cve_d = nc.dram_tensor("cve", [4, T], F32, kind="ExternalInput")
    out_d = nc.dram_tensor("out_t", [4, T], F32, kind="ExternalOutput")

    with SplitDrainTileContext(nc) as tc:
        with (
            tc.tile_pool(name="const", bufs=1) as cp,
            tc.tile_pool(name="vet", bufs=6) as vetp,
            tc.tile_pool(name="acts", bufs=2) as ap_,
            tc.tile_pool(name="outp", bufs=2) as op_,
            tc.tile_pool(name="psum", bufs=4, space="PSUM") as pp,
        ):
            # ---- persistent constants ----
            wv1 = cp.tile([VIEW_DIM, 256], BF)
            nc.scalar.dma_start(wv1[:], wv1_d[:])
            wv2 = cp.tile([128, 2, 2, 128], BF)
            nc.scalar.dma_start(wv2[:], wv2_d[:].rearrange("p (a b c) -> p a b c", a=2, b=2))
            wt1 = cp.tile([128, 2, 2, 128], VDT)
            nc.scalar.dma_start(wt1[:], wt1_d[:].rearrange("p (a b c) -> p a b c", a=2, b=2))
            wt2 = cp.tile([128, 2, 2, 128], VDT)
            nc.scalar.dma_start(wt2[:], wt2_d[:].rearrange("p (a b c) -> p a b c", a=2, b=2))
            wo = cp.tile([128, 4, 4], BF)
            nc.scalar.dma_start(wo[:], wo_d[:].rearrange("p (a b) -> p a b", a=4))
            # prefill output with the host-folded cve term; per-chunk output
            # DMAs are gpsimd software-DGE accum-adds on the same queue, so
            # ordering wrt this prefill is guaranteed.
            nc.gpsimd.dma_start(out_d[:, : (n_chunks - P) * CHUNK],
                                cve_d[:, : (n_chunks - P) * CHUNK])

            def relu_op(dst, src, eng):
                # eng 0 -> ACT, 1 -> DVE
                if eng == 0:
                    nc.scalar.activation(dst, src, AF.Relu)
                else:
                    nc.vector.tensor_scalar(dst, src, 0.0, None, op0=ALU.max)

            prev_out = [None]

            def out_chunk(g, pacts, c, cve_j=None):
                pj0 = g * P
                h2, v2 = pacts[c]["h2"], pacts[c]["v2"]
                rhs_tiles = [h2[:, 0, :], h2[:, 1, :], v2[:, 0, :], v2[:, 1, :]]
                pot = pp.tile([128, 2, CHUNK], F32, space="PSUM", tag="ps")
                po = pot[0:4, 0, :]
                for kt, rhs in enumerate(rhs_tiles):
                    nc.tensor.matmul(po, wo[:, kt, :], rhs,
                                     start=(kt == 0), stop=(kt == 3))
                ot = op_.tile([4, CHUNK], F32, tag="ot")
                if cve_j is not None:
                    # last group: fold cve on-engine, plain sync write (keeps
                    # the gpsimd SW-DGE queue empty well before the drain)
                    nc.vector.tensor_tensor(
                        ot[:], po, cve_j[:, c * CHUNK : (c + 1) * CHUNK],
                        op=ALU.add)
                    nc.sync.dma_start(
                        out_d[:, (pj0 + c) * CHUNK : (pj0 + c + 1) * CHUNK],
                        ot[:])
                    return
                if c % 2 == 0:
                    nc.vector.tensor_copy(ot[:], po)
                else:
                    nc.scalar.copy(ot[:], po)
                nc.gpsimd.dma_start(
                    out_d[:, (pj0 + c) * CHUNK : (pj0 + c + 1) * CHUNK],
                    ot[:], accum_op=ALU.add)

            for g in range(n_groups):
                j0 = g * P
                sc_j = vetp.tile([VIEW_DIM, P * CHUNK], BF, tag="scj")
                nc.sync.dma_start(sc_j[:], sc_d[:, j0 * CHUNK : (j0 + P) * CHUNK])
                veTs, acts = [], []
                for c in range(P):
                    veT = vetp.tile([128, 2, CHUNK], VDT, tag=f"veT{c}")
                    nc.sync.dma_start(veT[:], ve_d[j0 + c])
                    veTs.append(veT)
                    acts.append({})

                # ---- bf16 layer: one 2-bank psum tile per chunk ----
                def layer_bf(tag, wtile, rhs_of, ktiles, split_copy=False):
                    for c in range(P):
                        acts[c][tag] = ap_.tile([128, 2, CHUNK], BF,
                                                name=f"{tag}{c}", tag=f"{tag}{c}")
                        ps = pp.tile([128, 2, CHUNK], F32, space="PSUM", tag="ps")
                        for mt in range(2):
                            for kt in range(ktiles):
                                nc.tensor.matmul(
                                    ps[:, mt, :], wtile(kt, mt), rhs_of(c, kt),
                                    start=(kt == 0), stop=(kt == ktiles - 1))
                        if split_copy:
                            for mt in range(2):
                                relu_op(acts[c][tag][:, mt, :], ps[:, mt, :], mt)
                        else:
                            relu_op(acts[c][tag][:], ps[:], c % 2)

                # ---- fp8 DoubleRow layer: K=256 packed, N-tiles of 256 ----
                def layer_dr(tag, wtile, rhs_of, out_dtype, flip):
                    for c in range(P):
                        acts[c][tag] = ap_.tile([128, 2, CHUNK], out_dtype,
                                                name=f"{tag}{c}", tag=f"{tag}{c}")
                        ps = pp.tile([128, 2, CHUNK], F32, space="PSUM", tag="ps")
                        for mt in range(2):
                            for nt in range(2):
                                nc.tensor.matmul(
                                    ps[:, mt, nt * 256 : (nt + 1) * 256],
                                    wtile(mt),
                                    rhs_of(c)[:, :, nt * 256 : (nt + 1) * 256],
                                    start=True, stop=True, perf_mode=DR)
                        relu_op(acts[c][tag][:], ps[:], (c + flip) % 2)

                # v1(g) interleaved per-chunk with out(g-1): the out stage
                # fills the PE while v1's copies land, and vice versa
                for c in range(P):
                    acts[c]["v1"] = ap_.tile([128, 2, CHUNK], BF,
                                             name=f"v1{c}", tag=f"v1{c}")
                    ps = pp.tile([128, 2, CHUNK], F32, space="PSUM", tag="ps")
                    for mt in range(2):
                        nc.tensor.matmul(
                            ps[:, mt, :], wv1[:, mt * 128 : (mt + 1) * 128],
                            sc_j[:, c * CHUNK : (c + 1) * CHUNK],
                            start=True, stop=True)
                    for mt in range(2):
                        relu_op(acts[c]["v1"][:, mt, :], ps[:, mt, :], mt)
                    if prev_out[0] is not None:
                        out_chunk(g - 1, prev_out[0], c)
                layer_bf("v2", lambda kt, mt: wv2[:, kt, mt, :],
                         lambda c, kt: acts[c]["v1"][:, kt, :], 2)
                def h_chunk(tag, wt, rhs, out_dtype, c, eng):
                    acts[c][tag] = ap_.tile([128, 2, CHUNK], out_dtype,
                                            name=f"{tag}{c}", tag=f"{tag}{c}")
                    ps = pp.tile([128, 2, CHUNK], F32, space="PSUM", tag="ps")
                    for mt in range(2):
                        for nt in range(2):
                            nc.tensor.matmul(
                                ps[:, mt, nt * 256 : (nt + 1) * 256],
                                wt[:, :, mt, :],
                                rhs[:, :, nt * 256 : (nt + 1) * 256],
                                start=True, stop=True, perf_mode=DR)
                    relu_op(acts[c][tag][:], ps[:], eng)

                for c in range(P):
                    h_chunk("h1", wt1, veTs[c], FP8, c, (c + 1) % 2)
                for c in range(P):
                    h_chunk("h2", wt2, acts[c]["h1"], BF, c, c % 2)
                prev_out[0] = acts

            # last group's out stage, with cve added on-engine
            cve_l = op_.tile([4, P * CHUNK], F32, tag="cvel")
            nc.sync.dma_start(
                cve_l[:], cve_d[:, (n_chunks - P) * CHUNK : n_chunks * CHUNK])
            for c in range(P):
                out_chunk(n_groups - 1, prev_out[0], c, cve_j=cve_l)

    if split_waits:  # CoreSim can't run the raw NOPs; HW compile needs them
        _split_sync_waits(nc)
    return nc


# ---------------------------------------------------------------------------
# Host-side preprocessing
# ---------------------------------------------------------------------------

def _pack_w(w: np.ndarray) -> np.ndarray:
    """[256, 256] -> [128, 2*2*128] with layout [p, (kt, mt, j)]."""
    w4 = w.reshape(2, 128, 2, 128)           # [kt, p, mt, j]
    return np.ascontiguousarray(w4.transpose(1, 0, 2, 3)).reshape(128, 512)


def prepare_host_inputs(verts, barys, views, emb_tables, w_proj, b_proj,
                        view_W, view_b, vert_W, vert_b,
                        alpha_W1, alpha_b1, alpha_W2, alpha_b2,
                        color_W1, color_b1, color_W2, color_b2,
                        n_chunks=N_CHUNKS, n_cores=N_CORES):
    """Fold weights, gather+reduce embeddings, pack per-core in_maps."""
    verts = np.asarray(verts).astype(np.int64)
    barys = np.asarray(barys, dtype=np.float32)
    views = np.asarray(views, dtype=np.float32)
    emb = np.asarray(emb_tables, dtype=np.float32)

    t_core = n_chunks * CHUNK
    n_groups = n_chunks // P
    n_tok = t_core * n_cores

    # --- embedding tables: fold max_norm renorm ---
    norm = np.linalg.norm(emb.astype(np.float64), axis=-1, keepdims=True)
    scale = np.where(norm > 1.0, 1.0 / np.maximum(norm, 1e-7), 1.0)
    table = (emb * scale).reshape(VROWS, N_CHAN).astype(np.float32)

    # --- gather + barycentric reduce -> vertex embeddings [n_tok, 256] ---
    mesh_off = (np.arange(N_MESH, dtype=np.int64) * (N_VERTS + 1))[None, :, None]
    flat_idx = (verts + 1 + mesh_off).reshape(-1, 3)[:n_tok]
    flat_bary = barys.reshape(-1, 3)[:n_tok]
    vemb_f32 = np.einsum("tv,tvc->tc", flat_bary, table[flat_idx])
    vemb = vemb_f32.astype(E4)

    # --- sincos view features, transposed [36, n_tok] ---
    v64 = views.reshape(-1, 3).astype(np.float64)[:n_tok]
    freqs = 2.0 ** np.arange(N_LEVELS)
    xf = v64[:, None, :] * freqs[:, None]                 # [t, L, 3]
    sc = np.stack([np.sin(xf), np.cos(xf)], axis=2)       # [t, L, 2, 3]
    sc = sc.reshape(-1, VIEW_DIM).astype(np.float32)
    sc_T = np.ascontiguousarray(sc.T.astype(BF16))        # [36, n_tok]

    # --- folded weights (fp64) ---
    w_proj = np.asarray(w_proj, dtype=np.float64)
    b_proj = np.asarray(b_proj, dtype=np.float64)
    view_W = np.asarray(view_W, dtype=np.float64)
    view_b = np.asarray(view_b, dtype=np.float64)
    vert_W = np.asarray(vert_W, dtype=np.float64)
    vert_b = np.asarray(vert_b, dtype=np.float64)
    aW1 = np.asarray(alpha_W1, dtype=np.float64)
    ab1 = np.asarray(alpha_b1, dtype=np.float64)
    aW2 = np.asarray(alpha_W2, dtype=np.float64)
    ab2 = np.asarray(alpha_b2, dtype=np.float64)
    cW1 = np.asarray(color_W1, dtype=np.float64)
    cb1 = np.asarray(color_b1, dtype=np.float64)
    cW2 = np.asarray(color_W2, dtype=np.float64)
    cb2 = np.asarray(color_b2, dtype=np.float64)

    assert not np.any(b_proj) and not np.any(view_b) and not np.any(vert_b), \
        "kernel build assumes zero hidden biases (as in setup_inputs)"
    assert not np.any(ab1) and not np.any(cb1), \
        "kernel build assumes zero head hidden biases"

    wv1 = (w_proj @ view_W[0]).astype(BF16)               # [36, 256]
    wa = aW1 @ aW2                                        # [256, 1]
    ba = ab1 @ aW2 + ab2                                  # [1]
    wc = cW1 @ cW2                                        # [512, 3]
    bc = cb1 @ cW2 + cb2                                  # [3]

    w_out = np.zeros((512, 4), dtype=np.float64)
    w_out[0:256, 3] = wa[:, 0]        # h2 -> alpha
    w_out[256:512, 0:3] = wc[0:256]   # v2 -> colors
    wo = np.ascontiguousarray(
        w_out.reshape(4, 128, 4).transpose(1, 0, 2)).reshape(128, 16).astype(BF16)

    # host-folded output term: cve[t, 0:3] = ve @ Wc_bot + bc; cve[t, 3] = ba
    cve = np.empty((n_tok, 4), dtype=np.float32)
    cve[:, 0:3] = (vemb_f32.astype(np.float64) @ wc[256:512] + bc).astype(np.float32)
    cve[:, 3] = ba[0]

    shared = {
        "wv1": np.ascontiguousarray(wv1),
        "wv2": _pack_w(view_W[1]).astype(BF16),
        "wt1": _pack_w(vert_W[0]).astype(E4),
        "wt2": _pack_w(vert_W[1]).astype(E4),
        "wo": wo,
    }

    in_maps = []
    for c in range(n_cores):
        lo = c * t_core
        m = dict(shared)
        # [t_core, 256] -> [n_chunks, 128(chan%128), 2(half), 512(tok)]
        g = vemb[lo : lo + t_core].reshape(n_chunks, CHUNK, 2, 128)
        m["vet"] = np.ascontiguousarray(g.transpose(0, 3, 2, 1))
        m["sincos"] = np.ascontiguousarray(sc_T[:, lo : lo + t_core])
        m["cve"] = np.ascontiguousarray(cve[lo : lo + t_core].T)
        in_maps.append(m)
    return in_maps


def assemble_output(results, n_cores=N_CORES):
    """results[c]['out_t'] is [4, t_core] -> full (N_SAMPLES, N_MESH, 4)."""
    outs = []
    for c in range(n_cores):
        o = results[c]["out_t"]  # [4, t_core]
        outs.append(np.ascontiguousarray(o.T).reshape(-1, N_MESH, 4))
    return np.concatenate(outs, axis=0).astype(np.float32)


_NC_CACHE = {}


def get_nc(n_chunks=N_CHUNKS):
    if n_chunks not in _NC_CACHE:
        _NC_CACHE[n_chunks] = build_nc(n_chunks)
    return _NC_CACHE[n_chunks]


def kernel(**inputs) -> np.ndarray:
    in_maps = prepare_host_inputs(**inputs)
    nc = get_nc(N_CHUNKS)
    res = run_bass_kernel_spmd(nc, in_maps, list(range(N_CORES)))
    return assemble_output(res.results)


# revision 13
# speedup vs baseline: 1.2659x; 1.0205x over previous
"""MeshCaster Trainium2 kernel (v2: fp8 DoubleRow vert branch).

Per-token (token = (sample, mesh) pair, 262144 tokens) network:
  - gather 3 vertex embedding rows (per-mesh tables, max-norm renormalized)
  - barycentric weighted sum -> vertex embedding ve (256)
  - view branch: sincos(views) -> linear proj -> 2x (Linear+ReLU)
  - vert branch: 2x (Linear+ReLU)
  - alpha / color heads have identity activations.

Host-side folds (all exact linear algebra, fp64 weights):
  - max_norm renorm is a per-table-row property -> pre-scale tables
  - w_proj @ view_W[0] -> single [36 x 256] first view layer
  - alpha head:  (h@A1+b1)@A2+b2 = h@(A1@A2) + (b1@A2+b2)   [256x1]
  - color head:  (c@C1+b1)@C2+b2 = c@(C1@C2) + (b1@C2+b2)   [512x3]
  - alpha+color combine into one [768 x 4] output GEMM over [h2|v2|ve]
  - the gather + barycentric reduce (0.4% of FLOPs, pure data movement +
    a row-scale) run on host: the device's indirect-DMA descriptor
    generation path is ~1.7us per 128 rows on this toolchain, which would
    dominate the kernel. The device streams pre-reduced, channel-major ve
    tiles and executes all GEMMs (99.6% of the FLOPs).

v2 speedups over the 300us baseline:
  - vert branch (h1, h2) runs in fp8-e4m3 DoubleRow matmuls: 2 packed
    k-tiles per instruction at 2x rate (measured 118ns vs 225ns per
    equivalent bf16 pair). Only alpha (1 of 4 output channels) sees the
    quantization noise: full-net rel err 0.006 vs gate 2e-2.
  - ve streamed as fp8 (halves the dominant DMA stream).
  - 4-chunk interleave (P=4) with copies split ACT(mt0)/DVE(mt1) so
    PSUM->SBUF relu copies never stall the PE.
  - the host-folded cve term is pre-filled into the output DRAM buffer by
    a gpsimd DRAM->DRAM DMA; per-chunk outputs ride gpsimd software-DGE
    accum-add DMAs (same queue => ordered), with the PSUM->SBUF copy
    alternating between ACT and DVE. No vector-engine add needed.

Sharding: data-parallel over samples, 4096 samples (32768 tokens) per core,
weights replicated, no cross-core communication.

Device pipeline per 512-token chunk:
  v1 = relu(sincos[36,512] @ Wv1)        2 bf16 matmuls (K=36)
  v2 = relu(v1 @ Wv2)                    4 bf16 matmuls
  h1 = relu(veT @ Wt1)                   4 fp8 DoubleRow matmuls -> fp8
  h2 = relu(h1 @ Wt2)                    4 fp8 DoubleRow matmuls -> bf16
  out[4,512] = [h2|v2] @ Wo (+cve)       4 bf16 matmuls (psum-accumulated)
activations bf16/fp8, feature-major layout [chan, tok]; psum fp32.
"""

import sys

if "/opt/trn_rl_repo" not in sys.path:
    sys.path.insert(0, "/opt/trn_rl_repo")

import numpy as np
import ml_dtypes

import concourse.bass as bass
import concourse.tile as tile
from concourse import mybir
from concourse.bass_utils import run_bass_kernel_spmd
from concourse.vector_clock import ScopedClock

BF16 = ml_dtypes.bfloat16
E4 = ml_dtypes.float8_e4m3

N_SAMPLES = 32768
N_MESH = 8
N_VERTS = 50000
N_CHAN = 256
N_LEVELS = 6
VIEW_DIM = 3 * 2 * N_LEVELS  # 36
N_CORES = 8
VROWS = N_MESH * (N_VERTS + 1)  # 400008

T_CORE = (N_SAMPLES // N_CORES) * N_MESH  # 32768 tokens per core
CHUNK = 512
N_CHUNKS = T_CORE // CHUNK  # 64
P = 4                       # chunks interleaved per group
N_GROUPS = N_CHUNKS // P    # 16

F32 = mybir.dt.float32
BF = mybir.dt.bfloat16
FP8 = mybir.dt.float8e4
AF = mybir.ActivationFunctionType
ALU = mybir.AluOpType
DR = mybir.MatmulPerfMode.DoubleRow


class SplitDrainTileContext(tile.TileContext):
    """Walrus on this toolchain rejects >1 sync-wait on some instruction
    structs; split the kernel-tail drain's waits into single-wait NOPs."""

    def _drain_and_barrier(self, tick_clock, wait_clock):
        probe = self.nc.sync.nop(nofuse=True)
        wait_clock.add_sem_waits(probe.ins, ScopedClock({None: tick_clock.global_clock}))
        si = probe.ins.sync_info
        waits = list(si.on_wait) if si is not None else []
        if len(waits) > 1:
            si.on_wait = waits[:1]
            for w in waits[1:]:
                n = self.nc.sync.nop(nofuse=True)
                n.ins.sync_info = mybir.SyncInfo(on_wait=[w], on_update=[])
        self.nc.sync.drain()
        self.nc.all_engine_barrier()
        assert self.sems is not None
        popped = self.nc._tile_sem_poison_stack.pop()
        assert popped is self._sem_poison
        self.nc.clear_and_free_semaphores(list(self.sems.allocated().values()))
        self.nc.all_engine_barrier()


def _split_sync_waits(nc, max_waits=1):
    """Move excess per-instruction sync-waits onto same-engine NOPs."""
    cnt = 0
    for f in nc.m.functions:
        for bb in f.blocks:
            new = []
            for inst in bb.instructions:
                si = inst.sync_info
                if si is not None and len(si.on_wait) > max_waits:
                    waits = list(si.on_wait)
                    for w in waits[:-max_waits]:
                        cnt += 1
                        new.append(mybir.InstNoOp(
                            name=f"wsplit_{cnt}",
                            engine=inst.engine,
                            bass_nofuse=True,
                            sync_info=mybir.SyncInfo(on_wait=[w], on_update=[]),
                        ))
                    si.on_wait = waits[-max_waits:]
                new.append(inst)
            bb.instructions[:] = new
    return cnt


def build_nc(n_chunks: int, split_waits: bool = True) -> bass.Bass:
    """Build the Bass program for `n_chunks` 512-token chunks per core."""
    T = n_chunks * CHUNK
    n_groups = n_chunks // P
    nc = bass.Bass("TRN2", target_bir_lowering=False, debug=False)

    VDT = FP8
    # ---- DRAM I/O ----
    # channel-major vertex embeddings: [chunk, chan_in_half(128), half(2), tok(512)]
    ve_d = nc.dram_tensor("vet", [n_chunks, 128, 2, CHUNK], VDT, kind="ExternalInput")
    sc_d = nc.dram_tensor("sincos", [VIEW_DIM, T], BF, kind="ExternalInput")
    wv1_d = nc.dram_tensor("wv1", [VIEW_DIM, 256], BF, kind="ExternalInput")
    wv2_d = nc.dram_tensor("wv2", [128, 2 * 2 * 128], BF, kind="ExternalInput")
    wt1_d = nc.dram_tensor("wt1", [128, 2 * 2 * 128], VDT, kind="ExternalInput")
    wt2_d = nc.dram_tensor("wt2", [128, 2 * 2 * 128], VDT, kind="ExternalInput")
    wo_d = nc.dram_tensor("wo", [128, 2 * 4], BF, kind="ExternalInput")
    wa_d = nc.dram_tensor("wa", [128, 2 * 16], FP8, kind="ExternalInput")
    cve_d = nc.dram_tensor("cve", [4, T], F32, kind="ExternalInput")
    out_d = nc.dram_tensor("out_t", [4, T], F32, kind="ExternalOutput")

    with SplitDrainTileContext(nc) as tc:
        with (
            tc.tile_pool(name="const", bufs=1) as cp,
            tc.tile_pool(name="vet", bufs=6) as vetp,
            tc.tile_pool(name="acts", bufs=2) as ap_,
            tc.tile_pool(name="outp", bufs=2) as op_,
            tc.tile_pool(name="psum", bufs=4, space="PSUM") as pp,
        ):
            # ---- persistent constants ----
            wv1 = cp.tile([VIEW_DIM, 256], BF)
            nc.scalar.dma_start(wv1[:], wv1_d[:])
            wv2 = cp.tile([128, 2, 2, 128], BF)
            nc.scalar.dma_start(wv2[:], wv2_d[:].rearrange("p (a b c) -> p a b c", a=2, b=2))
            wt1 = cp.tile([128, 2, 2, 128], VDT)
            nc.scalar.dma_start(wt1[:], wt1_d[:].rearrange("p (a b c) -> p a b c", a=2, b=2))
            wt2 = cp.tile([128, 2, 2, 128], VDT)
            nc.scalar.dma_start(wt2[:], wt2_d[:].rearrange("p (a b c) -> p a b c", a=2, b=2))
            wo = cp.tile([128, 2, 4], BF)
            nc.scalar.dma_start(wo[:], wo_d[:].rearrange("p (a b) -> p a b", a=2))
            wa = cp.tile([128, 2, 16], FP8)
            nc.scalar.dma_start(wa[:], wa_d[:].rearrange("p (a b) -> p a b", a=2))
            # prefill output with the host-folded cve term; per-chunk output
            # DMAs are gpsimd software-DGE accum-adds on the same queue, so
            # ordering wrt this prefill is guaranteed.
            nc.gpsimd.dma_start(out_d[:, : (n_chunks - P) * CHUNK],
                                cve_d[:, : (n_chunks - P) * CHUNK])

            def relu_op(dst, src, eng):
                # eng 0 -> ACT, 1 -> DVE
                if eng == 0:
                    nc.scalar.activation(dst, src, AF.Relu)
                else:
                    nc.vector.tensor_scalar(dst, src, 0.0, None, op0=ALU.max)

            prev_out = [None]

            def out_chunk(g, pacts, c, cve_j=None):
                pj0 = g * P
                h2, v2 = pacts[c]["h2"], pacts[c]["v2"]
                pot = pp.tile([128, 2, CHUNK], F32, space="PSUM", tag="ps")
                po = pot[0:4, 0, :]
                # colors: bf16, writes rows 0-3 of bank0 (row 3 gets zeros)
                for kt in range(2):
                    nc.tensor.matmul(po, wo[:, kt, :], v2[:, kt, :],
                                     start=(kt == 0), stop=False,
                                     skip_group_check=True)
                # alpha: fp8 DR accumulating into the same region; the weight
                # sits in padded column 3, so row 3 += alpha, rows 0-2 += 0
                for nt in range(2):
                    nc.tensor.matmul(
                        pot[0:16, 0, nt * 256 : (nt + 1) * 256], wa[:],
                        h2[:, :, nt * 256 : (nt + 1) * 256],
                        start=False, stop=(nt == 1), perf_mode=DR,
                        skip_group_check=True)
                ot = op_.tile([4, CHUNK], F32, tag="ot")
                if cve_j is not None:
                    # last group: fold cve on-engine, plain sync write (keeps
                    # the gpsimd SW-DGE queue empty well before the drain)
                    nc.vector.tensor_tensor(
                        ot[:], po, cve_j[:, c * CHUNK : (c + 1) * CHUNK],
                        op=ALU.add)
                    nc.sync.dma_start(
                        out_d[:, (pj0 + c) * CHUNK : (pj0 + c + 1) * CHUNK],
                        ot[:])
                    return
                if c % 2 == 0:
                    nc.vector.tensor_copy(ot[:], po)
                else:
                    nc.scalar.copy(ot[:], po)
                nc.gpsimd.dma_start(
                    out_d[:, (pj0 + c) * CHUNK : (pj0 + c + 1) * CHUNK],
                    ot[:], accum_op=ALU.add)

            for g in range(n_groups):
                j0 = g * P
                sc_j = vetp.tile([VIEW_DIM, P * CHUNK], BF, tag="scj")
                nc.sync.dma_start(sc_j[:], sc_d[:, j0 * CHUNK : (j0 + P) * CHUNK])
                veTs, acts = [], []
                for c in range(P):
                    veT = vetp.tile([128, 2, CHUNK], VDT, tag=f"veT{c}")
                    nc.sync.dma_start(veT[:], ve_d[j0 + c])
                    veTs.append(veT)
                    acts.append({})

                # ---- bf16 layer: one 2-bank psum tile per chunk ----
                def layer_bf(tag, wtile, rhs_of, ktiles, split_copy=False):
                    for c in range(P):
                        acts[c][tag] = ap_.tile([128, 2, CHUNK], BF,
                                                name=f"{tag}{c}", tag=f"{tag}{c}")
                        ps = pp.tile([128, 2, CHUNK], F32, space="PSUM", tag="ps")
                        for mt in range(2):
                            for kt in range(ktiles):
                                nc.tensor.matmul(
                                    ps[:, mt, :], wtile(kt, mt), rhs_of(c, kt),
                                    start=(kt == 0), stop=(kt == ktiles - 1))
                        if split_copy:
                            for mt in range(2):
                                relu_op(acts[c][tag][:, mt, :], ps[:, mt, :], mt)
                        else:
                            relu_op(acts[c][tag][:], ps[:], c % 2)

                # ---- fp8 DoubleRow layer: K=256 packed, N-tiles of 256 ----
                def layer_dr(tag, wtile, rhs_of, out_dtype, flip):
                    for c in range(P):
                        acts[c][tag] = ap_.tile([128, 2, CHUNK], out_dtype,
                                                name=f"{tag}{c}", tag=f"{tag}{c}")
                        ps = pp.tile([128, 2, CHUNK], F32, space="PSUM", tag="ps")
                        for mt in range(2):
                            for nt in range(2):
                                nc.tensor.matmul(
                                    ps[:, mt, nt * 256 : (nt + 1) * 256],
                                    wtile(mt),
                                    rhs_of(c)[:, :, nt * 256 : (nt + 1) * 256],
                                    start=True, stop=True, perf_mode=DR)
                        relu_op(acts[c][tag][:], ps[:], (c + flip) % 2)

                # v1(g) interleaved per-chunk with out(g-1): the out stage
                # fills the PE while v1's copies land, and vice versa
                for c in range(P):
                    acts[c]["v1"] = ap_.tile([128, 2, CHUNK], BF,
                                             name=f"v1{c}", tag=f"v1{c}")
                    ps = pp.tile([128, 2, CHUNK], F32, space="PSUM", tag="ps")
                    for mt in range(2):
                        nc.tensor.matmul(
                            ps[:, mt, :], wv1[:, mt * 128 : (mt + 1) * 128],
                            sc_j[:, c * CHUNK : (c + 1) * CHUNK],
                            start=True, stop=True)
                    for mt in range(2):
                        relu_op(acts[c]["v1"][:, mt, :], ps[:, mt, :], mt)
                    if prev_out[0] is not None:
                        out_chunk(g - 1, prev_out[0], c)
                layer_bf("v2", lambda kt, mt: wv2[:, kt, mt, :],
                         lambda c, kt: acts[c]["v1"][:, kt, :], 2)
                def h_chunk(tag, wt, rhs, out_dtype, c, eng):
                    acts[c][tag] = ap_.tile([128, 2, CHUNK], out_dtype,
                                            name=f"{tag}{c}", tag=f"{tag}{c}")
                    ps = pp.tile([128, 2, CHUNK], F32, space="PSUM", tag="ps")
                    for mt in range(2):
                        for nt in range(2):
                            nc.tensor.matmul(
                                ps[:, mt, nt * 256 : (nt + 1) * 256],
                                wt[:, :, mt, :],
                                rhs[:, :, nt * 256 : (nt + 1) * 256],
                                start=True, stop=True, perf_mode=DR)
                    relu_op(acts[c][tag][:], ps[:], eng)

                for c in range(P):
                    h_chunk("h1", wt1, veTs[c], FP8, c, (c + 1) % 2)
                for c in range(P):
                    h_chunk("h2", wt2, acts[c]["h1"], FP8, c, c % 2)
                prev_out[0] = acts

            # last group's out stage, with cve added on-engine
            cve_l = op_.tile([4, P * CHUNK], F32, tag="cvel")
            nc.sync.dma_start(
                cve_l[:], cve_d[:, (n_chunks - P) * CHUNK : n_chunks * CHUNK])
            for c in range(P):
                out_chunk(n_groups - 1, prev_out[0], c, cve_j=cve_l)

    if split_waits:  # CoreSim can't run the raw NOPs; HW compile needs them
        _split_sync_waits(nc)
    return nc


# ---------------------------------------------------------------------------
# Host-side preprocessing
# ---------------------------------------------------------------------------

def _pack_w(w: np.ndarray) -> np.ndarray:
    """[256, 256] -> [128, 2*2*128] with layout [p, (kt, mt, j)]."""
    w4 = w.reshape(2, 128, 2, 128)           # [kt, p, mt, j]
    return np.ascontiguousarray(w4.transpose(1, 0, 2, 3)).reshape(128, 512)


def prepare_host_inputs(verts, barys, views, emb_tables, w_proj, b_proj,
                        view_W, view_b, vert_W, vert_b,
                        alpha_W1, alpha_b1, alpha_W2, alpha_b2,
                        color_W1, color_b1, color_W2, color_b2,
                        n_chunks=N_CHUNKS, n_cores=N_CORES):
    """Fold weights, gather+reduce embeddings, pack per-core in_maps."""
    verts = np.asarray(verts).astype(np.int64)
    barys = np.asarray(barys, dtype=np.float32)
    views = np.asarray(views, dtype=np.float32)
    emb = np.asarray(emb_tables, dtype=np.float32)

    t_core = n_chunks * CHUNK
    n_groups = n_chunks // P
    n_tok = t_core * n_cores

    # --- embedding tables: fold max_norm renorm ---
    norm = np.linalg.norm(emb.astype(np.float64), axis=-1, keepdims=True)
    scale = np.where(norm > 1.0, 1.0 / np.maximum(norm, 1e-7), 1.0)
    table = (emb * scale).reshape(VROWS, N_CHAN).astype(np.float32)

    # --- gather + barycentric reduce -> vertex embeddings [n_tok, 256] ---
    mesh_off = (np.arange(N_MESH, dtype=np.int64) * (N_VERTS + 1))[None, :, None]
    flat_idx = (verts + 1 + mesh_off).reshape(-1, 3)[:n_tok]
    flat_bary = barys.reshape(-1, 3)[:n_tok]
    vemb_f32 = np.einsum("tv,tvc->tc", flat_bary, table[flat_idx])
    vemb = vemb_f32.astype(E4)

    # --- sincos view features, transposed [36, n_tok] ---
    v64 = views.reshape(-1, 3).astype(np.float64)[:n_tok]
    freqs = 2.0 ** np.arange(N_LEVELS)
    xf = v64[:, None, :] * freqs[:, None]                 # [t, L, 3]
    sc = np.stack([np.sin(xf), np.cos(xf)], axis=2)       # [t, L, 2, 3]
    sc = sc.reshape(-1, VIEW_DIM).astype(np.float32)
    sc_T = np.ascontiguousarray(sc.T.astype(BF16))        # [36, n_tok]

    # --- folded weights (fp64) ---
    w_proj = np.asarray(w_proj, dtype=np.float64)
    b_proj = np.asarray(b_proj, dtype=np.float64)
    view_W = np.asarray(view_W, dtype=np.float64)
    view_b = np.asarray(view_b, dtype=np.float64)
    vert_W = np.asarray(vert_W, dtype=np.float64)
    vert_b = np.asarray(vert_b, dtype=np.float64)
    aW1 = np.asarray(alpha_W1, dtype=np.float64)
    ab1 = np.asarray(alpha_b1, dtype=np.float64)
    aW2 = np.asarray(alpha_W2, dtype=np.float64)
    ab2 = np.asarray(alpha_b2, dtype=np.float64)
    cW1 = np.asarray(color_W1, dtype=np.float64)
    cb1 = np.asarray(color_b1, dtype=np.float64)
    cW2 = np.asarray(color_W2, dtype=np.float64)
    cb2 = np.asarray(color_b2, dtype=np.float64)

    assert not np.any(b_proj) and not np.any(view_b) and not np.any(vert_b), \
        "kernel build assumes zero hidden biases (as in setup_inputs)"
    assert not np.any(ab1) and not np.any(cb1), \
        "kernel build assumes zero head hidden biases"

    wv1 = (w_proj @ view_W[0]).astype(BF16)               # [36, 256]
    wa = aW1 @ aW2                                        # [256, 1]
    ba = ab1 @ aW2 + ab2                                  # [1]
    wc = cW1 @ cW2                                        # [512, 3]
    bc = cb1 @ cW2 + cb2                                  # [3]

    w_col = np.zeros((256, 4), dtype=np.float64)
    w_col[:, 0:3] = wc[0:256]         # v2 -> colors; col 3 stays zero
    wo = np.ascontiguousarray(
        w_col.reshape(2, 128, 4).transpose(1, 0, 2)).reshape(128, 8).astype(BF16)
    wa8 = np.zeros((128, 2, 16), dtype=np.float64)
    wa8[:, 0, 3] = wa[0:128, 0]       # h2 -> alpha, padded col 3 (16B align)
    wa8[:, 1, 3] = wa[128:256, 0]
    wa8 = wa8.reshape(128, 32).astype(E4)

    # host-folded output term: cve[t, 0:3] = ve @ Wc_bot + bc; cve[t, 3] = ba
    cve = np.empty((n_tok, 4), dtype=np.float32)
    cve[:, 0:3] = (vemb_f32.astype(np.float64) @ wc[256:512] + bc).astype(np.float32)
    cve[:, 3] = ba[0]

    shared = {
        "wv1": np.ascontiguousarray(wv1),
        "wv2": _pack_w(view_W[1]).astype(BF16),
        "wt1": _pack_w(vert_W[0]).astype(E4),
        "wt2": _pack_w(vert_W[1]).astype(E4),
        "wo": wo,
        "wa": wa8,
    }

    in_maps = []
    for c in range(n_cores):
        lo = c * t_core
        m = dict(shared)
        # [t_core, 256] -> [n_chunks, 128(chan%128), 2(half), 512(tok)]
        g = vemb[lo : lo + t_core].reshape(n_chunks, CHUNK, 2, 128)
        m["vet"] = np.ascontiguousarray(g.transpose(0, 3, 2, 1))
        m["sincos"] = np.ascontiguousarray(sc_T[:, lo : lo + t_core])
        m["cve"] = np.ascontiguousarray(cve[lo : lo + t_core].T)
        in_maps.append(m)
    return in_maps


def assemble_output(results, n_cores=N_CORES):
    """results[c]['out_t'] is [4, t_core] -> full (N_SAMPLES, N_MESH, 4)."""
    outs = []
    for c in range(n_cores):
        o = results[c]["out_t"]  # [4, t_core]
        outs.append(np.ascontiguousarray(o.T).reshape(-1, N_MESH, 4))
    return np.concatenate(outs, axis=0).astype(np.float32)


_NC_CACHE = {}


def get_nc(n_chunks=N_CHUNKS):
    if n_chunks not in _NC_CACHE:
        _NC_CACHE[n_chunks] = build_nc(n_chunks)
    return _NC_CACHE[n_chunks]


def kernel(**inputs) -> np.ndarray:
    in_maps = prepare_host_inputs(**inputs)
    nc = get_nc(N_CHUNKS)
    res = run_bass_kernel_spmd(nc, in_maps, list(range(N_CORES)))
    return assemble_output(res.results)
